# revision 1
# baseline (speedup 1.0000x reference)
"""Cost-volume kernel for Trainium2 (8 NeuronCores, batch-parallel).

out[b, k, h, w] = (1/(C*81)) * sum_c x[b,c,h,w] * warped[b,c,h+di,w+dj]
for the 81 offsets (di,dj) in [-4,4]^2 (zero-padded), B=8 -> one batch
element per core.

Device-side algorithm (per core):
  - the image is tiled into 8x16 x-tiles (16x16 = 256 tiles). For each
    tile one TensorE matmul computes ALL pairwise channel-dot-products
    between the 128 x-positions (lhsT [C=128, M=128]) and the 16x24
    zero-padded warped window (rhs [C=128, N=384]) -> PSUM [128, 384].
  - warped is staged per tile-COLUMN as SBUF strips [C, 136, 24] (halo
    columns duplicated) so every tile's rhs window is one contiguous
    384-element slice (matmul operands require single-free-dim APs).
  - PSUM blocks are scaled by 1/(C*81) and copied to SBUF (ACT/DVE
    alternating), then DMA'd to a DRAM scratch output [128, 256*384].
  - The 81 shifted dot products per position sit on constant-stride
    "diagonals" of these blocks; on-chip engines cannot express
    partition-correlated free offsets, so the final relayout to
    [81, H, W] is a pure constant-stride (as_strided) view applied while
    unsharding on the host.
"""

import numpy as np

B = 8
C, H, W = 128, 128, 256
R = 4
K = 2 * R + 1  # 9
NOFF = K * K  # 81
TH, TW = 8, 16  # x-tile shape (M = TH*TW = 128)
NH, NW = TH + 2 * R, TW + 2 * R  # warped window 16 x 24
N = NH * NW  # 384
SCALE = 1.0 / (C * NOFF)

# "bf16": bf16 matmul operands + bf16 dump (fp32 PSUM accumulation);
#         halves HBM traffic and avoids the fp32 hi/lo double-pass on PE.
# "f32": exact fp32 end-to-end.
PRECISION = "bf16"

_CACHE = {}


def _build_module(c, h, w, th, tw, r, n_cores, precision):
    import concourse.bacc as bacc
    import concourse.mybir as mybir
    import concourse.tile as tile

    k = 2 * r + 1
    nh, nw = th + 2 * r, tw + 2 * r
    n = nh * nw
    nt_h, nt_w = h // th, w // tw
    ntiles = nt_h * nt_w
    ph = h + 2 * r  # padded strip rows
    scale = 1.0 / (c * k * k)
    dt = mybir.dt.float32 if precision == "f32" else mybir.dt.bfloat16
    f32 = mybir.dt.float32

    nc = bacc.Bacc(
        "TRN2", target_bir_lowering=False, debug=False, num_devices=n_cores
    )
    # Inputs are pre-stripped host-side (see _host_prep): per tile-column
    # contiguous blocks, warped pre-padded with its 4-wide zero halo and
    # duplicated halo columns. All device DMAs are fully contiguous.
    x_d = nc.dram_tensor("x", [c, nt_w, h * tw], dt, kind="ExternalInput").ap()
    w_d = nc.dram_tensor("warped", [c, nt_w, ph * nw], dt, kind="ExternalInput").ap()
    out_d = nc.dram_tensor("dump", [128, ntiles * n], dt, kind="ExternalOutput").ap()

    with tile.TileContext(nc) as tc:
        # PSUM drain is batched: G matmuls land in one bank-padded PSUM
        # group (each MM within its own 2KB bank), drained by a single
        # strided ACT/DVE copy to amortize per-op overhead.
        G = 4 if nt_h % 4 == 0 else (2 if nt_h % 2 == 0 else 1)
        BANK = 512  # fp32 elements per PSUM bank
        assert n <= BANK
        with (
            tc.tile_pool(name="wstrip", bufs=4) as ws_pool,
            tc.tile_pool(name="xstrip", bufs=4) as x_pool,
            tc.tile_pool(name="dump", bufs=8) as dump_pool,
            tc.tile_pool(name="psum", bufs=2, space="PSUM") as psum_pool,
        ):
            t = 0
            gidx = 0
            # loads ride the SP (warped) and ACT (x) HWDGE rings; stores
            # round-robin across SWDGE/SP/ACT so transfers overlap.
            store_engines = [nc.gpsimd, nc.sync, nc.scalar]
            assert nt_h % G == 0
            for itw in range(nt_w):
                ws = ws_pool.tile([128, ph * nw], dt)
                nc.sync.dma_start(out=ws, in_=w_d[:, itw, :])
                xs = x_pool.tile([128, h * tw], dt)
                nc.scalar.dma_start(out=xs, in_=x_d[:, itw, :])
                ps = None
                for ith in range(nt_h):
                    if ith % G == 0:
                        ps = psum_pool.tile([128, G * BANK], f32)
                    lhsT = xs[:, ith * th * tw : (ith + 1) * th * tw]
                    rhs = ws[:, ith * th * nw : ith * th * nw + n]
                    j = ith % G
                    nc.tensor.matmul(
                        ps[:, j * BANK : j * BANK + n], lhsT, rhs,
                        start=True, stop=True,
                    )
                    t += 1
                    if j == G - 1:
                        # one strided drain for the G tiles -> its own dump
                        # tile, which ships immediately (small stores keep
                        # the buffer pipeline loose)
                        db = dump_pool.tile([128, G * n], dt)
                        src = ps[:].rearrange("p (g x) -> p g x", g=G)[:, :, 0:n]
                        dst = db[:].rearrange("p (g x) -> p g x", g=G)
                        if gidx % 2 == 0:
                            nc.scalar.mul(dst, src, scale)
                        else:
                            nc.vector.tensor_scalar_mul(dst, src, scale)
                        gidx += 1
                        eng = store_engines[gidx % len(store_engines)]
                        base = (itw * nt_h + ith - G + 1) * n
                        eng.dma_start(
                            out=out_d[:, base : base + G * n], in_=db
                        )
            assert t == ntiles

    nc.compile()
    return nc


def _host_prep(x_b, warped_b, h, w, th, tw, r):
    """Re-layout one batch element into per-tile-column strips.

    x: [c, h, w] -> [c, nt_w, h*tw]   (column strips, h-major)
    warped:      -> [c, nt_w, ph*nw]  (pre-padded strips with halo cols)
    """
    c = x_b.shape[0]
    nh, nw = th + 2 * r, tw + 2 * r
    ph = h + 2 * r
    nt_w = w // tw
    xs = np.ascontiguousarray(
        x_b.reshape(c, h, nt_w, tw).transpose(0, 2, 1, 3)
    ).reshape(c, nt_w, h * tw)
    wp = np.zeros((c, ph, w + 2 * r), dtype=x_b.dtype)
    wp[:, r : r + h, r : r + w] = warped_b
    sc, sh, sw = wp.strides
    strips = np.lib.stride_tricks.as_strided(
        wp, shape=(c, nt_w, ph, nw), strides=(sc, tw * sw, sh, sw)
    )
    ws = np.ascontiguousarray(strips).reshape(c, nt_w, ph * nw)
    return xs, ws


def _extract(dump, h, w, th, tw, r):
    """[128, ntiles*n] f32 scratch -> [81, h, w] via constant-stride view.

    Tile order is tw-major: t = itw*nt_h + ith.
    """
    k = 2 * r + 1
    nh, nw = th + 2 * r, tw + 2 * r
    n = nh * nw
    nt_h, nt_w = h // th, w // tw
    ntiles = nt_h * nt_w
    dmp = np.ascontiguousarray(dump).reshape(128, ntiles, n)
    sm, st, sn = dmp.strides
    # element [m=(hx*tw+wx), t=(itw*nt_h+ith), n=((hx+di)*nw + wx+dj)]
    view = np.lib.stride_tricks.as_strided(
        dmp,
        shape=(k, k, nt_h, th, nt_w, tw),
        strides=(
            nw * sn,            # di
            sn,                 # dj
            st,                 # ith
            tw * sm + nw * sn,  # hx
            nt_h * st,          # itw
            sm + sn,            # wx
        ),
    )
    return np.ascontiguousarray(view).reshape(k * k, h, w).astype(np.float32)


def kernel(x, warped):
    from concourse import bass_utils

    x = np.asarray(x, dtype=np.float32)
    warped = np.asarray(warped, dtype=np.float32)
    assert x.shape == (B, C, H, W) and warped.shape == (B, C, H, W)

    if PRECISION == "bf16":
        import ml_dtypes

        x = x.astype(ml_dtypes.bfloat16)
        warped = warped.astype(ml_dtypes.bfloat16)

    key = PRECISION
    if key not in _CACHE:
        _CACHE[key] = _build_module(C, H, W, TH, TW, R, B, PRECISION)
    nc = _CACHE[key]

    in_maps = []
    for b in range(B):
        xs, ws = _host_prep(x[b], warped[b], H, W, TH, TW, R)
        in_maps.append({"x": xs, "warped": ws})
    res = bass_utils.run_bass_kernel_spmd(nc, in_maps, core_ids=list(range(B)))
    global LAST_RESULTS
    LAST_RESULTS = res
    out = np.empty((B, NOFF, H, W), dtype=np.float32)
    for b in range(B):
        out[b] = _extract(res.results[b]["dump"], H, W, TH, TW, R)
    return out



# revision 4
# speedup vs baseline: 1.3038x; 1.3038x over previous
"""Cost-volume kernel for Trainium2 (8 NeuronCores, batch-parallel).

out[b, k, h, w] = (1/(C*81)) * sum_c x[b,c,h,w] * warped[b,c,h+di,w+dj]
for the 81 offsets (di,dj) in [-4,4]^2 (zero-padded), B=8 -> one batch
element per core.

Device-side algorithm (per core), v3 "col-tiled":
  - the image is tiled into 4x8 x-tiles (M=32). One PSUM block [128,192]
    holds 4 adjacent tiles (same tile-row, 4 consecutive tile-cols) via
    4 PE *column-tiled* matmuls (tile_position=(0,32j)) that run
    concurrently on independent 128x32 sub-arrays. Each matmul:
    lhsT = x-tile [C=128, 32] (tile-contiguous x layout, 1D AP — the
    stationary operand must be single-free-dim), rhs = the tile's
    12x16 window of the zero-padded warped image as a 2D-strided AP
    [C, 12, 16] (verified supported for the moving operand).
  - this shrinks the per-position PSUM footprint from 384 (8x16 tiles)
    to 192 values -> the DRAM dump halves to 12.6 MB/core.
  - warped is staged as 4 full-width row-bands [C, 40*264] of the
    padded image (8-row halo overlap, 1.18x dup vs 1.5x for column
    strips); x as 4 tile-contiguous bands.
  - PSUM pool tiles of [128, 1024] (2 banks) hold 4 blocks (offsets
    0/192/512/704 f32 to stay bank-internal); one strided ACT/DVE
    scaled copy drains 4 blocks -> SBUF bf16 [128, 768] -> DMA store.
  - final relayout [81, H, W] is a constant-stride view on the host.
"""

import numpy as np

B = 8
C, H, W = 128, 128, 256
R = 4
K = 2 * R + 1  # 9
NOFF = K * K  # 81
TH, TW = 4, 8  # x-tile shape (M = 32)
NH, NW = TH + 2 * R, TW + 2 * R  # window 12 x 16
N = NH * NW  # 192
SCALE = 1.0 / (C * NOFF)

NT_H, NT_W = H // TH, W // TW  # 32 x 32 tile grid
TPB = 4  # tiles per PSUM block (4 col-tiles)
BPG = 4  # blocks per drain group (2 PSUM banks)
TR_BAND = 8  # tile-rows per band (32 image rows)
NBANDS = NT_H // TR_BAND  # 4
PW = W + 2 * R  # 264 padded cols
BROWS = TR_BAND * TH + 2 * R  # 40 padded rows per warped band
BLOCKS_PER_BAND = TR_BAND * (NT_W // TPB)  # 8*8 = 64
GROUPS_PER_BAND = BLOCKS_PER_BAND // BPG  # 16
NGROUPS = NBANDS * GROUPS_PER_BAND  # 64

PRECISION = "bf16"

_CACHE = {}


def _build_module(n_cores):
    import concourse.bacc as bacc
    import concourse.mybir as mybir
    import concourse.tile as tile

    dt = mybir.dt.bfloat16
    f32 = mybir.dt.float32
    # f32 offsets of the 4 blocks inside a [128, 1024] (2-bank) psum tile:
    # blocks 0,1 -> bank0 at 0/192; blocks 2,3 -> bank1 at 512/704.
    BLK_OFF = [0, 192, 512, 704]

    nc = bacc.Bacc(
        "TRN2", target_bir_lowering=False, debug=False, num_devices=n_cores
    )
    # x: tile-contiguous [C, nt_h, nt_w, TH*TW]; warped: padded row-major
    # [C, 136, 264]. Both host-prepped so every DMA is fully contiguous.
    x_d = nc.dram_tensor(
        "x", [C, NT_H * NT_W * TH * TW], dt, kind="ExternalInput"
    ).ap()
    w_d = nc.dram_tensor(
        "warped", [C, (H + 2 * R) * PW], dt, kind="ExternalInput"
    ).ap()
    out_d = nc.dram_tensor(
        "dump", [128, NGROUPS * BPG * N], dt, kind="ExternalOutput"
    ).ap()

    x_band_elems = TR_BAND * NT_W * TH * TW  # 8192
    w_band_elems = BROWS * PW  # 10560

    with tile.TileContext(nc) as tc:
        with (
            tc.tile_pool(name="wband", bufs=2) as wb_pool,
            tc.tile_pool(name="xband", bufs=2) as xb_pool,
            tc.tile_pool(name="dump", bufs=8) as dump_pool,
            tc.tile_pool(name="psum", bufs=4, space="PSUM") as psum_pool,
        ):
            store_engines = [nc.gpsimd, nc.sync, nc.scalar]
            gidx = 0
            for band in range(NBANDS):
                wbase = band * TR_BAND * TH * PW
                wsb = wb_pool.tile([128, w_band_elems], dt)
                nc.sync.dma_start(
                    out=wsb, in_=w_d[:, wbase : wbase + w_band_elems]
                )
                xsb = xb_pool.tile([128, x_band_elems], dt)
                nc.scalar.dma_start(
                    out=xsb,
                    in_=x_d[:, band * x_band_elems : (band + 1) * x_band_elems],
                )
                w2 = wsb[:].rearrange("p (h w) -> p h w", w=PW)
                blk = 0
                ps = None
                for ltr in range(TR_BAND):  # tile-row within band
                    for q in range(NT_W // TPB):  # block of 4 tile-cols
                        s = blk % BPG
                        if s == 0:
                            ps = psum_pool.tile([128, 1024], f32)
                        for j in range(TPB):
                            itw = q * TPB + j
                            xoff = (ltr * NT_W + itw) * (TH * TW)
                            lhsT = xsb[:, xoff : xoff + TH * TW]
                            rhs = w2[
                                :,
                                ltr * TH : ltr * TH + NH,
                                itw * TW : itw * TW + NW,
                            ]
                            nc.tensor.matmul(
                                ps[32 * j : 32 * (j + 1),
                                   BLK_OFF[s] : BLK_OFF[s] + N],
                                lhsT,
                                rhs,
                                start=True,
                                stop=True,
                                tile_position=(0, 32 * j),
                            )
                        blk += 1
                        if s == BPG - 1:
                            db = dump_pool.tile([128, BPG * N], dt)
                            src = ps[:].rearrange(
                                "p (b x) -> p b x", b=2
                            )[:, :, 0 : 2 * N]
                            dst = db[:].rearrange("p (b x) -> p b x", b=2)
                            if gidx % 2 == 0:
                                nc.scalar.mul(dst, src, SCALE)
                            else:
                                nc.vector.tensor_scalar_mul(dst, src, SCALE)
                            eng = store_engines[gidx % len(store_engines)]
                            eng.dma_start(
                                out=out_d[
                                    :, gidx * BPG * N : (gidx + 1) * BPG * N
                                ],
                                in_=db,
                            )
                            gidx += 1
            assert gidx == NGROUPS

    nc.compile()
    return nc


def _host_prep(x_b, warped_b):
    """x: [c,h,w] -> tile-contiguous [c, nt_h*nt_w*32]; warped -> padded
    row-major [c, 136*264]."""
    c = x_b.shape[0]
    xt = np.ascontiguousarray(
        x_b.reshape(c, NT_H, TH, NT_W, TW).transpose(0, 1, 3, 2, 4)
    ).reshape(c, NT_H * NT_W * TH * TW)
    wp = np.zeros((c, H + 2 * R, PW), dtype=x_b.dtype)
    wp[:, R : R + H, R : R + W] = warped_b
    return xt, wp.reshape(c, (H + 2 * R) * PW)


def _extract(dump):
    """[128, NGROUPS*BPG*N] -> [81, H, W] constant-stride view.

    dump element [m, g, s*N + n]:
      m = 32*j + hx*TW + wx ; n = (hx+di)*NW + (wx+dj)
      g = band*16 + ltr*2 + q//4 ; s = q%4
      h = band*32 + ltr*4 + hx ;  w = (q*4+j)*8 + wx
    """
    dmp = np.ascontiguousarray(dump).reshape(128, NGROUPS, BPG * N)
    sm, sg, sn = dmp.strides
    sn_e = sn  # innermost element stride (bytes)
    view = np.lib.stride_tricks.as_strided(
        dmp,
        shape=(K, K, NBANDS, TR_BAND, TH, 2, 4, TPB, TW),
        #      di dj band   ltr     hx  qa qb  j   wx
        strides=(
            NW * sn_e,            # di
            sn_e,                 # dj
            16 * sg,              # band
            2 * sg,               # ltr
            TW * sm + NW * sn_e,  # hx
            sg,                   # qa = q//4
            N * sn_e,             # qb = q%4 (= s)
            32 * sm,              # j
            sm + sn_e,            # wx
        ),
    )
    # [di,dj, band,ltr,hx, qa,qb,j,wx] -> [81, H, W]
    out = np.ascontiguousarray(view).reshape(NOFF, H, W)
    return out.astype(np.float32)


def kernel(x, warped):
    from concourse import bass_utils

    x = np.asarray(x, dtype=np.float32)
    warped = np.asarray(warped, dtype=np.float32)
    assert x.shape == (B, C, H, W) and warped.shape == (B, C, H, W)

    import ml_dtypes

    x = x.astype(ml_dtypes.bfloat16)
    warped = warped.astype(ml_dtypes.bfloat16)

    key = "v3"
    if key not in _CACHE:
        _CACHE[key] = _build_module(B)
    nc = _CACHE[key]

    in_maps = []
    for b in range(B):
        xt, wp = _host_prep(x[b], warped[b])
        in_maps.append({"x": xt, "warped": wp})
    res = bass_utils.run_bass_kernel_spmd(nc, in_maps, core_ids=list(range(B)))
    global LAST_RESULTS
    LAST_RESULTS = res
    out = np.empty((B, NOFF, H, W), dtype=np.float32)
    for b in range(B):
        out[b] = _extract(res.results[b]["dump"])
    return out


# revision 10
# speedup vs baseline: 1.3224x; 1.0142x over previous
"""Cost-volume kernel for Trainium2 (8 NeuronCores, batch-parallel).

out[b, k, h, w] = (1/(C*81)) * sum_c x[b,c,h,w] * warped[b,c,h+di,w+dj]
for the 81 offsets (di,dj) in [-4,4]^2 (zero-padded), B=8 -> one batch
element per core.

Device-side algorithm (per core), v3 "col-tiled":
  - the image is tiled into 4x8 x-tiles (M=32). One PSUM block [128,192]
    holds 4 adjacent tiles (same tile-row, 4 consecutive tile-cols) via
    4 PE *column-tiled* matmuls (tile_position=(0,32j)) that run
    concurrently on independent 128x32 sub-arrays. Each matmul:
    lhsT = x-tile [C=128, 32] (tile-contiguous x layout, 1D AP — the
    stationary operand must be single-free-dim), rhs = the tile's
    12x16 window of the zero-padded warped image as a 2D-strided AP
    [C, 12, 16] (verified supported for the moving operand).
  - this shrinks the per-position PSUM footprint from 384 (8x16 tiles)
    to 192 values -> the DRAM dump halves to 12.6 MB/core.
  - warped is staged as 4 full-width row-bands [C, 40*264] of the
    padded image (8-row halo overlap, 1.18x dup vs 1.5x for column
    strips); x as 4 tile-contiguous bands.
  - PSUM pool tiles of [128, 1024] (2 banks) hold 4 blocks (offsets
    0/192/512/704 f32 to stay bank-internal); one strided ACT/DVE
    scaled copy drains 4 blocks -> SBUF bf16 [128, 768] -> DMA store.
  - final relayout [81, H, W] is a constant-stride view on the host.
"""

import numpy as np

B = 8
C, H, W = 128, 128, 256
R = 4
K = 2 * R + 1  # 9
NOFF = K * K  # 81
TH, TW = 4, 8  # x-tile shape (M = 32)
NH, NW = TH + 2 * R, TW + 2 * R  # window 12 x 16
N = NH * NW  # 192
SCALE = 1.0 / (C * NOFF)

NT_H, NT_W = H // TH, W // TW  # 32 x 32 tile grid
TPB = 4  # tiles per PSUM block (4 col-tiles)
BPG = 4  # blocks per drain group (2 PSUM banks)
PW = W + 2 * R  # 264 padded cols
# Non-overlapping warped row-bands (no halo re-read). Windows that
# straddle a band edge are computed by TWO partial-window matmuls
# writing disjoint PSUM column ranges.
W_EDGES = [0, 36, 68, 100, 136]
X_TRB = 8  # tile-rows per x band
NGROUPS = NT_H * (NT_W // TPB) // BPG  # 64

PRECISION = "bf16"

_CACHE = {}


def _build_module(n_cores):
    import concourse.bacc as bacc
    import concourse.mybir as mybir
    import concourse.tile as tile

    dt = mybir.dt.bfloat16
    f32 = mybir.dt.float32
    # f32 offsets of the 4 blocks inside a [128, 1024] (2-bank) psum tile:
    # blocks 0,1 -> bank0 at 0/192; blocks 2,3 -> bank1 at 512/704.
    BLK_OFF = [0, 192, 512, 704]

    nc = bacc.Bacc(
        "TRN2", target_bir_lowering=False, debug=False, num_devices=n_cores
    )
    # x: tile-contiguous [C, nt_h, nt_w, TH*TW]; warped: padded row-major
    # [C, 136, 264]. Both host-prepped so every DMA is fully contiguous.
    x_d = nc.dram_tensor(
        "x", [C, NT_H * NT_W * TH * TW], dt, kind="ExternalInput"
    ).ap()
    w_d = nc.dram_tensor(
        "warped", [C, (H + 2 * R) * PW], dt, kind="ExternalInput"
    ).ap()
    out_d = nc.dram_tensor(
        "dump", [128, NGROUPS * BPG * N], dt, kind="ExternalOutput"
    ).ap()

    x_band_elems = X_TRB * NT_W * TH * TW  # 8192

    with tile.TileContext(nc) as tc:
        with (
            tc.tile_pool(name="wband", bufs=3) as wb_pool,
            tc.tile_pool(name="xband", bufs=3) as xb_pool,
            tc.tile_pool(name="dump", bufs=8) as dump_pool,
            tc.tile_pool(name="psum", bufs=4, space="PSUM") as psum_pool,
        ):
            store_engines = [nc.gpsimd, nc.sync, nc.scalar]
            # warped band tiles, loaded lazily in tile-row order
            wtiles = [None] * (len(W_EDGES) - 1)
            w2s = [None] * (len(W_EDGES) - 1)

            def get_wband(b):
                if wtiles[b] is None:
                    lo, hi = W_EDGES[b], W_EDGES[b + 1]
                    t = wb_pool.tile([128, (hi - lo) * PW], dt)
                    nc.sync.dma_start(out=t, in_=w_d[:, lo * PW : hi * PW])
                    wtiles[b] = t
                    w2s[b] = t[:].rearrange("p (h w) -> p h w", w=PW)
                return w2s[b]

            xtiles = [None] * (NT_H // X_TRB)

            def get_xband(b):
                if xtiles[b] is None:
                    t = xb_pool.tile([128, x_band_elems], dt)
                    nc.scalar.dma_start(
                        out=t,
                        in_=x_d[:, b * x_band_elems : (b + 1) * x_band_elems],
                    )
                    xtiles[b] = t
                return xtiles[b]

            gidx = 0
            blk = 0
            ps = None
            for ltr in range(NT_H):  # global tile-row
                r0 = ltr * TH  # first padded window row
                # band(s) covering rows [r0, r0+NH)
                b0 = max(i for i in range(len(W_EDGES) - 1) if W_EDGES[i] <= r0)
                split = r0 + NH > W_EDGES[b0 + 1]
                xsb = get_xband(ltr // X_TRB)
                # prefetch bands needed by the NEXT tile-row (one row lead)
                if ltr + 1 < NT_H:
                    rn = (ltr + 1) * TH
                    bn = max(
                        i for i in range(len(W_EDGES) - 1) if W_EDGES[i] <= rn
                    )
                    get_wband(bn)
                    if rn + NH > W_EDGES[bn + 1]:
                        get_wband(bn + 1)
                    get_xband((ltr + 1) // X_TRB)
                for q in range(NT_W // TPB):
                    s = blk % BPG
                    if s == 0:
                        ps = psum_pool.tile([128, 1024], f32)
                    for j in range(TPB):
                        itw = q * TPB + j
                        xoff = ((ltr % X_TRB) * NT_W + itw) * (TH * TW)
                        lhsT = xsb[:, xoff : xoff + TH * TW]
                        pj = ps[32 * j : 32 * (j + 1), :]
                        if not split:
                            w2 = get_wband(b0)
                            rhs = w2[
                                :,
                                r0 - W_EDGES[b0] : r0 - W_EDGES[b0] + NH,
                                itw * TW : itw * TW + NW,
                            ]
                            nc.tensor.matmul(
                                pj[:, BLK_OFF[s] : BLK_OFF[s] + N],
                                lhsT, rhs, start=True, stop=True,
                                tile_position=(0, 32 * j),
                            )
                        else:
                            edge = W_EDGES[b0 + 1]
                            ra = edge - r0  # rows from band b0
                            w2a = get_wband(b0)
                            w2b = get_wband(b0 + 1)
                            rhs_a = w2a[
                                :,
                                r0 - W_EDGES[b0] : edge - W_EDGES[b0],
                                itw * TW : itw * TW + NW,
                            ]
                            rhs_b = w2b[
                                :, 0 : r0 + NH - edge,
                                itw * TW : itw * TW + NW,
                            ]
                            nc.tensor.matmul(
                                pj[:, BLK_OFF[s] : BLK_OFF[s] + ra * NW],
                                lhsT, rhs_a, start=True, stop=True,
                                tile_position=(0, 32 * j),
                            )
                            nc.tensor.matmul(
                                pj[:, BLK_OFF[s] + ra * NW : BLK_OFF[s] + N],
                                lhsT, rhs_b, start=True, stop=True,
                                tile_position=(0, 32 * j),
                            )
                    blk += 1
                    if s == BPG - 1:
                        db = dump_pool.tile([128, BPG * N], dt)
                        src = ps[:].rearrange(
                            "p (b x) -> p b x", b=2
                        )[:, :, 0 : 2 * N]
                        dst = db[:].rearrange("p (b x) -> p b x", b=2)
                        if gidx % 2 == 0:
                            nc.scalar.mul(dst, src, SCALE)
                        else:
                            nc.vector.tensor_scalar_mul(dst, src, SCALE)
                        eng = store_engines[gidx % len(store_engines)]
                        eng.dma_start(
                            out=out_d[
                                :, gidx * BPG * N : (gidx + 1) * BPG * N
                            ],
                            in_=db,
                        )
                        gidx += 1
            assert gidx == NGROUPS

    nc.compile()
    return nc


def _host_prep(x_b, warped_b):
    """x: [c,h,w] -> tile-contiguous [c, nt_h*nt_w*32]; warped -> padded
    row-major [c, 136*264]."""
    c = x_b.shape[0]
    xt = np.ascontiguousarray(
        x_b.reshape(c, NT_H, TH, NT_W, TW).transpose(0, 1, 3, 2, 4)
    ).reshape(c, NT_H * NT_W * TH * TW)
    wp = np.zeros((c, H + 2 * R, PW), dtype=x_b.dtype)
    wp[:, R : R + H, R : R + W] = warped_b
    return xt, wp.reshape(c, (H + 2 * R) * PW)


def _extract(dump):
    """[128, NGROUPS*BPG*N] -> [81, H, W] constant-stride view.

    dump element [m, g, s*N + n]:
      m = 32*j + hx*TW + wx ; n = (hx+di)*NW + (wx+dj)
      g = ltr*2 + q//4 ; s = q%4
      h = ltr*4 + hx ;  w = (q*4+j)*8 + wx
    """
    dmp = np.ascontiguousarray(dump).reshape(128, NGROUPS, BPG * N)
    sm, sg, sn = dmp.strides
    sn_e = sn  # innermost element stride (bytes)
    view = np.lib.stride_tricks.as_strided(
        dmp,
        shape=(K, K, NT_H, TH, 2, 4, TPB, TW),
        #      di dj ltr   hx  qa qb  j   wx
        strides=(
            NW * sn_e,            # di
            sn_e,                 # dj
            2 * sg,               # ltr
            TW * sm + NW * sn_e,  # hx
            sg,                   # qa = q//4
            N * sn_e,             # qb = q%4 (= s)
            32 * sm,              # j
            sm + sn_e,            # wx
        ),
    )
    # [di,dj, ltr,hx, qa,qb,j,wx] -> [81, H, W]
    out = np.ascontiguousarray(view).reshape(NOFF, H, W)
    return out.astype(np.float32)


def kernel(x, warped):
    from concourse import bass_utils

    x = np.asarray(x, dtype=np.float32)
    warped = np.asarray(warped, dtype=np.float32)
    assert x.shape == (B, C, H, W) and warped.shape == (B, C, H, W)

    import ml_dtypes

    x = x.astype(ml_dtypes.bfloat16)
    warped = warped.astype(ml_dtypes.bfloat16)

    key = "v3"
    if key not in _CACHE:
        _CACHE[key] = _build_module(B)
    nc = _CACHE[key]

    in_maps = []
    for b in range(B):
        xt, wp = _host_prep(x[b], warped[b])
        in_maps.append({"x": xt, "warped": wp})
    res = bass_utils.run_bass_kernel_spmd(nc, in_maps, core_ids=list(range(B)))
    global LAST_RESULTS
    LAST_RESULTS = res
    out = np.empty((B, NOFF, H, W), dtype=np.float32)
    for b in range(B):
        out[b] = _extract(res.results[b]["dump"])
    return out


# revision 11
# speedup vs baseline: 1.4388x; 1.0881x over previous
"""Cost-volume kernel for Trainium2 (8 NeuronCores, batch-parallel).

out[b, k, h, w] = (1/(C*81)) * sum_c x[b,c,h,w] * warped[b,c,h+di,w+dj]
for the 81 offsets (di,dj) in [-4,4]^2 (zero-padded), B=8 -> one batch
element per core.

Device-side algorithm (per core), v3 "col-tiled":
  - the image is tiled into 4x8 x-tiles (M=32). One PSUM block [128,192]
    holds 4 adjacent tiles (same tile-row, 4 consecutive tile-cols) via
    4 PE *column-tiled* matmuls (tile_position=(0,32j)) that run
    concurrently on independent 128x32 sub-arrays. Each matmul:
    lhsT = x-tile [C=128, 32] (tile-contiguous x layout, 1D AP — the
    stationary operand must be single-free-dim), rhs = the tile's
    12x16 window of the zero-padded warped image as a 2D-strided AP
    [C, 12, 16] (verified supported for the moving operand).
  - this shrinks the per-position PSUM footprint from 384 (8x16 tiles)
    to 192 values -> the DRAM dump halves to 12.6 MB/core.
  - warped is staged as 4 full-width row-bands [C, 40*264] of the
    padded image (8-row halo overlap, 1.18x dup vs 1.5x for column
    strips); x as 4 tile-contiguous bands.
  - PSUM pool tiles of [128, 1024] (2 banks) hold 4 blocks (offsets
    0/192/512/704 f32 to stay bank-internal); one strided ACT/DVE
    scaled copy drains 4 blocks -> SBUF bf16 [128, 768] -> DMA store.
  - final relayout [81, H, W] is a constant-stride view on the host.
"""

import numpy as np

B = 8
C, H, W = 128, 128, 256
R = 4
K = 2 * R + 1  # 9
NOFF = K * K  # 81
TH, TW = 4, 8  # x-tile shape (M = 32)
NH, NW = TH + 2 * R, TW + 2 * R  # window 12 x 16
N = NH * NW  # 192
SCALE = 1.0 / (C * NOFF)

NT_H, NT_W = H // TH, W // TW  # 32 x 32 tile grid
TPB = 4  # tiles per PSUM block (4 col-tiles)
BPG = 8  # blocks per drain group (4 PSUM banks)
PW = W + 2 * R  # 264 padded cols
# Non-overlapping warped row-bands (no halo re-read). Windows that
# straddle a band edge are computed by TWO partial-window matmuls
# writing disjoint PSUM column ranges.
W_EDGES = [0, 36, 68, 100, 136]
X_TRB = 8  # tile-rows per x band
NGROUPS = NT_H * (NT_W // TPB) // BPG  # 32

PRECISION = "bf16"

_CACHE = {}


def _build_module(n_cores):
    import concourse.bacc as bacc
    import concourse.mybir as mybir
    import concourse.tile as tile

    dt = mybir.dt.bfloat16
    f32 = mybir.dt.float32
    # f32 offsets of the 8 blocks inside a [128, 2048] (4-bank) psum
    # tile: 2 blocks per 512-f32 bank at bank-internal 0/192.
    BLK_OFF = [(s // 2) * 512 + (s % 2) * 192 for s in range(8)]

    nc = bacc.Bacc(
        "TRN2", target_bir_lowering=False, debug=False, num_devices=n_cores
    )
    # x: tile-contiguous [C, nt_h, nt_w, TH*TW]; warped: padded row-major
    # [C, 136, 264]. Both host-prepped so every DMA is fully contiguous.
    x_d = nc.dram_tensor(
        "x", [C, NT_H * NT_W * TH * TW], dt, kind="ExternalInput"
    ).ap()
    w_d = nc.dram_tensor(
        "warped", [C, (H + 2 * R) * PW], dt, kind="ExternalInput"
    ).ap()
    out_d = nc.dram_tensor(
        "dump", [128, NGROUPS * BPG * N], dt, kind="ExternalOutput"
    ).ap()

    x_band_elems = X_TRB * NT_W * TH * TW  # 8192

    with tile.TileContext(nc) as tc:
        with (
            tc.tile_pool(name="wband", bufs=3) as wb_pool,
            tc.tile_pool(name="xband", bufs=3) as xb_pool,
            tc.tile_pool(name="dump", bufs=8) as dump_pool,
            tc.tile_pool(name="psum", bufs=2, space="PSUM") as psum_pool,
        ):
            store_engines = [nc.sync, nc.scalar, nc.gpsimd]
            # warped band tiles, loaded lazily in tile-row order
            wtiles = [None] * (len(W_EDGES) - 1)
            w2s = [None] * (len(W_EDGES) - 1)

            def get_wband(b):
                if wtiles[b] is None:
                    lo, hi = W_EDGES[b], W_EDGES[b + 1]
                    t = wb_pool.tile([128, (hi - lo) * PW], dt)
                    nc.sync.dma_start(out=t, in_=w_d[:, lo * PW : hi * PW])
                    wtiles[b] = t
                    w2s[b] = t[:].rearrange("p (h w) -> p h w", w=PW)
                return w2s[b]

            xtiles = [None] * (NT_H // X_TRB)

            def get_xband(b):
                if xtiles[b] is None:
                    t = xb_pool.tile([128, x_band_elems], dt)
                    nc.scalar.dma_start(
                        out=t,
                        in_=x_d[:, b * x_band_elems : (b + 1) * x_band_elems],
                    )
                    xtiles[b] = t
                return xtiles[b]

            gidx = 0
            blk = 0
            ps = None
            for ltr in range(NT_H):  # global tile-row
                r0 = ltr * TH  # first padded window row
                # band(s) covering rows [r0, r0+NH)
                b0 = max(i for i in range(len(W_EDGES) - 1) if W_EDGES[i] <= r0)
                split = r0 + NH > W_EDGES[b0 + 1]
                xsb = get_xband(ltr // X_TRB)
                # prefetch bands needed by the NEXT tile-row (one row lead)
                if ltr + 1 < NT_H:
                    rn = (ltr + 1) * TH
                    bn = max(
                        i for i in range(len(W_EDGES) - 1) if W_EDGES[i] <= rn
                    )
                    get_wband(bn)
                    if rn + NH > W_EDGES[bn + 1]:
                        get_wband(bn + 1)
                    get_xband((ltr + 1) // X_TRB)
                for q in range(NT_W // TPB):
                    s = blk % BPG
                    if s == 0:
                        ps = psum_pool.tile([128, 2048], f32)
                    for j in range(TPB):
                        itw = q * TPB + j
                        xoff = ((ltr % X_TRB) * NT_W + itw) * (TH * TW)
                        lhsT = xsb[:, xoff : xoff + TH * TW]
                        pj = ps[32 * j : 32 * (j + 1), :]
                        if not split:
                            w2 = get_wband(b0)
                            rhs = w2[
                                :,
                                r0 - W_EDGES[b0] : r0 - W_EDGES[b0] + NH,
                                itw * TW : itw * TW + NW,
                            ]
                            nc.tensor.matmul(
                                pj[:, BLK_OFF[s] : BLK_OFF[s] + N],
                                lhsT, rhs, start=True, stop=True,
                                tile_position=(0, 32 * j),
                            )
                        else:
                            edge = W_EDGES[b0 + 1]
                            ra = edge - r0  # rows from band b0
                            w2a = get_wband(b0)
                            w2b = get_wband(b0 + 1)
                            rhs_a = w2a[
                                :,
                                r0 - W_EDGES[b0] : edge - W_EDGES[b0],
                                itw * TW : itw * TW + NW,
                            ]
                            rhs_b = w2b[
                                :, 0 : r0 + NH - edge,
                                itw * TW : itw * TW + NW,
                            ]
                            nc.tensor.matmul(
                                pj[:, BLK_OFF[s] : BLK_OFF[s] + ra * NW],
                                lhsT, rhs_a, start=True, stop=True,
                                tile_position=(0, 32 * j),
                            )
                            nc.tensor.matmul(
                                pj[:, BLK_OFF[s] + ra * NW : BLK_OFF[s] + N],
                                lhsT, rhs_b, start=True, stop=True,
                                tile_position=(0, 32 * j),
                            )
                    blk += 1
                    if s == BPG - 1:
                        db = dump_pool.tile([128, BPG * N], dt)
                        src = ps[:].rearrange(
                            "p (b x) -> p b x", b=4
                        )[:, :, 0 : 2 * N]
                        dst = db[:].rearrange("p (b x) -> p b x", b=4)
                        if gidx % 2 == 0:
                            nc.scalar.mul(dst, src, SCALE)
                        else:
                            nc.vector.tensor_scalar_mul(dst, src, SCALE)
                        eng = store_engines[gidx % len(store_engines)]
                        eng.dma_start(
                            out=out_d[
                                :, gidx * BPG * N : (gidx + 1) * BPG * N
                            ],
                            in_=db,
                        )
                        gidx += 1
            assert gidx == NGROUPS

    nc.compile()
    return nc


def _host_prep(x_b, warped_b):
    """x: [c,h,w] -> tile-contiguous [c, nt_h*nt_w*32]; warped -> padded
    row-major [c, 136*264]."""
    c = x_b.shape[0]
    xt = np.ascontiguousarray(
        x_b.reshape(c, NT_H, TH, NT_W, TW).transpose(0, 1, 3, 2, 4)
    ).reshape(c, NT_H * NT_W * TH * TW)
    wp = np.zeros((c, H + 2 * R, PW), dtype=x_b.dtype)
    wp[:, R : R + H, R : R + W] = warped_b
    return xt, wp.reshape(c, (H + 2 * R) * PW)


def _extract(dump):
    """[128, NGROUPS*BPG*N] -> [81, H, W] constant-stride view.

    dump element [m, g, s*N + n]:
      m = 32*j + hx*TW + wx ; n = (hx+di)*NW + (wx+dj)
      g = ltr ; s = q
      h = ltr*4 + hx ;  w = (q*4+j)*8 + wx
    """
    dmp = np.ascontiguousarray(dump).reshape(128, NGROUPS, BPG * N)
    sm, sg, sn = dmp.strides
    sn_e = sn  # innermost element stride (bytes)
    view = np.lib.stride_tricks.as_strided(
        dmp,
        shape=(K, K, NT_H, TH, 8, TPB, TW),
        #      di dj ltr   hx  q  j   wx
        strides=(
            NW * sn_e,            # di
            sn_e,                 # dj
            sg,                   # ltr
            TW * sm + NW * sn_e,  # hx
            N * sn_e,             # q (= s)
            32 * sm,              # j
            sm + sn_e,            # wx
        ),
    )
    # [di,dj, ltr,hx, q,j,wx] -> [81, H, W]
    out = np.ascontiguousarray(view).reshape(NOFF, H, W)
    return out.astype(np.float32)


def kernel(x, warped):
    from concourse import bass_utils

    x = np.asarray(x, dtype=np.float32)
    warped = np.asarray(warped, dtype=np.float32)
    assert x.shape == (B, C, H, W) and warped.shape == (B, C, H, W)

    import ml_dtypes

    x = x.astype(ml_dtypes.bfloat16)
    warped = warped.astype(ml_dtypes.bfloat16)

    key = "v3"
    if key not in _CACHE:
        _CACHE[key] = _build_module(B)
    nc = _CACHE[key]

    in_maps = []
    for b in range(B):
        xt, wp = _host_prep(x[b], warped[b])
        in_maps.append({"x": xt, "warped": wp})
    res = bass_utils.run_bass_kernel_spmd(nc, in_maps, core_ids=list(range(B)))
    global LAST_RESULTS
    LAST_RESULTS = res
    out = np.empty((B, NOFF, H, W), dtype=np.float32)
    for b in range(B):
        out[b] = _extract(res.results[b]["dump"])
    return out
